# revision 1
# baseline (speedup 1.0000x reference)
"""MixtureOfExpertsTreeEnsemble Trainium2 kernel (8-core SPMD, batch data-parallel).

Math (per batch row b, tree t):
  g[b,n,t] = sigmoid(x[b] @ W[n,:,t] + bias[n,t])          63 internal nodes
  p[b,l,t] = prod of g / (1-g) along root->leaf path        64 leaves
  w[l,d,t] = leaf_weight[l,d,t] * softmax_t(gates[l,d,t])
  out[b,d] = sum_{l,t} p[b,l,t] * w[l,d,t]

Sharding: batch 4096 -> 8 cores x 512 rows; node weights / leaf tables are
replicated (small).  No collectives; host concatenates the per-core outputs.

Device-side structure (per core):
  * all streamed operands are bf16 (the gates are bf16 downstream anyway, so
    f32 logits precision would be wasted); PSUM accumulation stays f32
  * phase A (PE):   logits as [b_tile(128), (node,tree)] bf16 matmuls.
    Loop is (btile-pair, chunk): each (node,tree) chunk is consumed as soon
    as its DMA lands, and one [128,1008] 2-bank PSUM tile serves two batch
    tiles -> half the sigmoid instructions on ACT
  * phase B (DVE):  leaf path probabilities by level doubling in a *block*
    layout (children stored [left | right]) so every op is contiguous and
    bf16 (DVE 2x mode); host pre-permutes node order (bit-reversal within
    each level) and the leaf tables to match
  * phase C (PE):   p transposed to [(leaf,tree), b] bf16 chunks, 4 chunks
    per PSUM bank, one DVE copy per bank
  * phase 0:        w = leaf_weight * softmax(gates) with (l,d) on partitions
    and t free: exp on ACT, reduce on DVE, the 64 normalize ops on the idle
    Pool engine; PE transposes are emitted after the batch loop so they
    never block phase A on the leaf-table DMAs
  * phase D (PE):   out_T[d,b] = sum_chunks w_T.T @ p_T, host transposes back
  * DMA: weight matrix on the SP HW-DGE ring; x / leaf tables / output on the
    ACT ring so the two streams overlap
"""

import sys

sys.path.insert(0, "/opt/trn_rl_repo")

import ml_dtypes
import numpy as np

BF16 = np.dtype(ml_dtypes.bfloat16)

MAX_DEPTH = 6
NUM_TREES = 64
LEAF_DIMS = 128
D_IN = 512
BATCH = 4096
N_INTERNAL = 63
N_LEAVES = 64
N_CORES = 8
BS = BATCH // N_CORES          # 512 batch rows per core
KT = D_IN // 128               # 4 contraction tiles
NT = N_INTERNAL * NUM_TREES    # 4032 (node,tree) pairs
NCHUNK = 8
CHUNK = NT // NCHUNK           # 504
NBT = BS // 128                # 4 batch tiles per core
NPT = N_LEAVES * NUM_TREES // 128  # 32 transpose chunks of (leaf,tree)


def _bitrev(x: int, bits: int) -> int:
    r = 0
    for _ in range(bits):
        r = (r << 1) | (x & 1)
        x >>= 1
    return r


# block-recursion orderings (see module docstring)
_NODES_PERM = np.array(
    [(2**lvl - 1) + _bitrev(j, lvl) for lvl in range(MAX_DEPTH) for j in range(2**lvl)]
)
_LEAF_PERM = np.array([_bitrev(j, MAX_DEPTH) for j in range(N_LEAVES)])

_BUILT = {}


def _build(use_bias: bool):
    """Build + finalize the per-core Bass program."""
    import concourse.bacc as bacc
    import concourse.tile as tile
    from concourse import mybir
    from concourse.masks import make_identity

    f32 = mybir.dt.float32
    f32r = mybir.dt.float32r
    bf16 = mybir.dt.bfloat16
    AF = mybir.ActivationFunctionType
    AX = mybir.AxisListType
    MUL = mybir.AluOpType.mult

    nc = bacc.Bacc("TRN2", target_bir_lowering=False, debug=False)

    xT = nc.dram_tensor("xT", [KT, 128, BS], bf16, kind="ExternalInput")
    Wf = nc.dram_tensor("Wf", [KT, 128, NT], bf16, kind="ExternalInput")
    # leaf tables host-transposed to [d, (leaf, tree)] for contiguous DMA
    gt = nc.dram_tensor("gt", [LEAF_DIMS, N_LEAVES * NUM_TREES], bf16, kind="ExternalInput")
    lwt = nc.dram_tensor("lwt", [LEAF_DIMS, N_LEAVES * NUM_TREES], bf16, kind="ExternalInput")
    if use_bias:
        bias = nc.dram_tensor("bias", [1, NT], bf16, kind="ExternalInput")
    outT = nc.dram_tensor("outT", [LEAF_DIMS, BS], f32, kind="ExternalOutput")

    with tile.TileContext(nc) as tc:
        with tc.tile_pool(name="const", bufs=1) as cpool, \
             tc.tile_pool(name="wts", bufs=1) as wpool, \
             tc.tile_pool(name="psA", bufs=4, space="PSUM") as psA, \
             tc.tile_pool(name="psT", bufs=3, space="PSUM") as psT, \
             tc.tile_pool(name="psO", bufs=1, space="PSUM") as psO:

            ident = cpool.tile([128, 128], bf16, tag="ident")
            make_identity(nc, ident[:])

            # ---- input DMAs.  x + leaf tables on the ACT HW-DGE ring, the
            # (bigger) weight matrix on the SP ring, so they overlap. ----
            xk = []
            for k in range(KT):
                t = wpool.tile([128, BS], bf16, tag=f"xk{k}", name=f"xk{k}")
                nc.scalar.dma_start(t[:], xT[k, :, :])
                xk.append(t)

            wk = [wpool.tile([128, NT], bf16, tag=f"wk{k}", name=f"wk{k}") for k in range(KT)]
            for m in range(NCHUNK // 2):
                for k in range(KT):
                    nc.sync.dma_start(
                        wk[k][:, m * 2 * CHUNK:(m + 1) * 2 * CHUNK],
                        Wf[k, :, m * 2 * CHUNK:(m + 1) * 2 * CHUNK],
                    )
            if use_bias:
                bias_sb = cpool.tile([1, NT], bf16, tag="bias")
                nc.sync.dma_start(bias_sb[:], bias[:, :])
                ones1 = cpool.tile([1, 128], bf16, tag="ones1")
                nc.gpsimd.memset(ones1[:], 1.0)

            wsm = wpool.tile([128, N_LEAVES, NUM_TREES], bf16, tag="wsm")
            gtile = wpool.tile([128, N_LEAVES, NUM_TREES], bf16, tag="gtile")
            nc.sync.dma_start(gtile[:], gt[:, :].rearrange("d (l t) -> d l t", t=NUM_TREES))
            lwtile = wpool.tile([128, N_LEAVES, NUM_TREES], bf16, tag="lwtile")
            nc.sync.dma_start(lwtile[:], lwt[:, :].rearrange("d (l t) -> d l t", t=NUM_TREES))

            def emit_phase0_compute():
                # w = leaf_weight * softmax(gates): exp on ACT, reduce on DVE,
                # normalize on the idle Pool engine (PE transposes deferred)
                nc.scalar.activation(gtile[:], gtile[:], AF.Exp)
                ehalf = cpool.tile([128, N_LEAVES, NUM_TREES // 2], bf16, tag="ehalf")
                nc.vector.tensor_add(ehalf[:], gtile[:, :, 0:NUM_TREES // 2],
                                     gtile[:, :, NUM_TREES // 2:NUM_TREES])
                equar = cpool.tile([128, N_LEAVES, NUM_TREES // 4], bf16, tag="equar")
                nc.vector.tensor_add(equar[:], ehalf[:, :, 0:NUM_TREES // 4],
                                     ehalf[:, :, NUM_TREES // 4:NUM_TREES // 2])
                s = cpool.tile([128, N_LEAVES], f32, tag="s")
                nc.vector.reduce_sum(s[:], equar[:], axis=AX.X)
                r = cpool.tile([128, N_LEAVES], f32, tag="r")
                nc.vector.reciprocal(r[:], s[:])
                for l in range(N_LEAVES):
                    nc.vector.scalar_tensor_tensor(
                        wsm[:, l, :], gtile[:, l, :], r[:, l:l + 1], lwtile[:, l, :],
                        op0=MUL, op1=MUL,
                    )

            # ---- main loop: batch-tile pairs ----
            with tc.tile_pool(name="gp", bufs=1) as gpool, \
                 tc.tile_pool(name="pp", bufs=2) as ppool, \
                 tc.tile_pool(name="pfp", bufs=2) as pfpool, \
                 tc.tile_pool(name="ptp", bufs=1) as pTpool, \
                 tc.tile_pool(name="outp", bufs=1) as outpool:

                out_ps = psO.tile([LEAF_DIMS, BS], f32, tag="out_ps")
                out_sb = outpool.tile([LEAF_DIMS, BS], f32, tag="out_sb")

                # PE warm-up: dummy transposes into the (not yet used) output
                # PSUM bank while the first weight DMAs are in flight, so the
                # HAM clock gate is released before phase A starts
                ident32 = cpool.tile([128, 128], f32, tag="ident32")
                make_identity(nc, ident32[:])
                for _wi in range(10):
                    nc.tensor.transpose(out_ps[:, 0:128], ident32[:], ident32[:])

                def emit_phaseD_slice(i, width=1):
                    bsl = slice(i * 128, (i + width) * 128)
                    for j in range(NPT):
                        nc.tensor.matmul(out_ps[:, bsl],
                                         wT_all[:, j // 4, (j % 4) * 128:(j % 4 + 1) * 128],
                                         pT_all[:, j, bsl],
                                         start=(j == 0), stop=(j == NPT - 1))
                    nc.vector.tensor_copy(out_sb[:, bsl], out_ps[:, bsl])
                    nc.sync.dma_start(outT[:, bsl], out_sb[:, bsl])

                # pT_all[:, j, :] = chunk j of p_T, [(leaf,tree)(128), b(512)]
                pT_all = pTpool.tile([128, NPT, BS], bf16, tag="pT")
                # g_all[:, i, :] = sigmoid gates for batch tile i
                g_all = gpool.tile([128, NBT, NT], bf16, tag="g")
                wT_all = wpool.tile([128, NPT // 4, 512], bf16, tag="wT")

                for pair in range(NBT // 2):
                    i0 = 2 * pair
                    # phase A: one 2-bank PSUM tile serves both batch tiles of
                    # the pair; chunks consumed in DMA arrival order
                    for q in range(2):
                        bsl = slice((i0 + q) * 128, (i0 + q + 1) * 128)
                        for n in range(NCHUNK):
                            csl = slice(n * CHUNK, (n + 1) * CHUNK)
                            lg = psA.tile([128, CHUNK], f32, tag="lg")
                            for k in range(KT):
                                nc.tensor.matmul(
                                    lg[:], xk[k][:, bsl], wk[k][:, csl],
                                    start=(k == 0),
                                    stop=(k == KT - 1 and not use_bias),
                                )
                            if use_bias:
                                nc.tensor.matmul(
                                    lg[:], ones1[:], bias_sb[:, csl],
                                    start=False, stop=True,
                                )
                            nc.scalar.activation(g_all[:, i0 + q, csl], lg[:], AF.Sigmoid)

                    if pair == 1:
                        # phase 0 PE part here: wsm is ready by now and this
                        # keeps the post-loop PE tail short
                        for jj in range(NPT // 4):
                            tp4 = psT.tile([128, 512], bf16, tag="tp")
                            for qq in range(4):
                                j = 4 * jj + qq
                                nc.tensor.transpose(
                                    tp4[:, qq * 128:(qq + 1) * 128],
                                    wsm[:, 2 * j:2 * j + 2, :], ident[:])
                            nc.scalar.copy(wT_all[:, jj, :], tp4[:])

                        emit_phaseD_slice(0, width=2)

                    for q in range(2):
                        i = i0 + q
                        bsl = slice(i * 128, (i + 1) * 128)
                        # phase B: block-layout level doubling (DVE, 2x mode)
                        pa = ppool.tile([128, 2048], bf16, tag="pa")
                        pb = ppool.tile([128, 2048], bf16, tag="pb")
                        pf = pfpool.tile([128, 4096], bf16, tag="pf")
                        # level 0: p = [g0 | 1-g0]
                        nc.vector.tensor_copy(pa[:, 0:64], g_all[:, i, 0:64])
                        nc.scalar.activation(pa[:, 64:128], g_all[:, i, 0:64],
                                             AF.Copy, bias=1.0, scale=-1.0)
                        cur, other = pa, pb
                        for lvl in range(1, MAX_DEPTH):
                            h = (2 ** lvl) * 64
                            off = (2 ** lvl - 1) * 64
                            dst = pf if lvl == MAX_DEPTH - 1 else other
                            if lvl == MAX_DEPTH - 1:
                                # half-split so the first transpose chunks can
                                # start while the second half still computes
                                hh = h // 2
                                nc.vector.tensor_mul(dst[:, 0:hh], cur[:, 0:hh],
                                                     g_all[:, i, off:off + hh])
                                nc.vector.tensor_sub(dst[:, h:h + hh], cur[:, 0:hh],
                                                     dst[:, 0:hh])
                                nc.vector.tensor_mul(dst[:, hh:h], cur[:, hh:h],
                                                     g_all[:, i, off + hh:off + h])
                                nc.vector.tensor_sub(dst[:, h + hh:2 * h], cur[:, hh:h],
                                                     dst[:, hh:h])
                            else:
                                nc.vector.tensor_mul(dst[:, 0:h], cur[:, 0:h],
                                                     g_all[:, i, off:off + h])
                                nc.vector.tensor_sub(dst[:, h:2 * h], cur[:, 0:h],
                                                     dst[:, 0:h])
                            cur, other = dst, cur

                        # phase C: transpose p -> [(leaf,tree), b] bf16;
                        # 4 chunks share one PSUM bank, one DVE copy per bank
                        for jj in range(NPT // 4):
                            tp4 = psT.tile([128, 512], bf16, tag="tp")
                            for qq in range(4):
                                j = 4 * jj + qq
                                nc.tensor.transpose(
                                    tp4[:, qq * 128:(qq + 1) * 128],
                                    pf[:, j * 128:(j + 1) * 128], ident[:])
                            mod = 2 if pair == 1 else 3
                            if jj % mod == mod - 1:
                                nc.scalar.copy(
                                    pT_all[:, 4 * jj:4 * jj + 4, bsl], tp4[:])
                            else:
                                nc.vector.tensor_copy(
                                    pT_all[:, 4 * jj:4 * jj + 4, bsl], tp4[:])

                        if pair == 1:
                            emit_phaseD_slice(i)

                    if pair == 0:
                        emit_phase0_compute()



    nc.finalize()
    return nc


def _get_nc(use_bias: bool):
    if use_bias not in _BUILT:
        _BUILT[use_bias] = _build(use_bias)
    return _BUILT[use_bias]


def _make_in_maps(x, W, b, leaf_weight, gates):
    x = np.ascontiguousarray(np.asarray(x, dtype=np.float32))
    W = np.asarray(W, dtype=np.float32)
    b = np.asarray(b, dtype=np.float32)
    leaf_weight = np.asarray(leaf_weight, dtype=np.float32)
    gates = np.asarray(gates, dtype=np.float32)

    use_bias = bool(np.any(b))
    # host-side layout prep (permutations / transposes / bf16 cast)
    Wp = W[_NODES_PERM]                                   # [63, 512, 64]
    Wf = np.ascontiguousarray(
        Wp.transpose(1, 0, 2).reshape(KT, 128, NT).astype(BF16))
    # leaf tables -> [d, (leaf, tree)] so the DMA is contiguous per partition
    gt = np.ascontiguousarray(
        gates[_LEAF_PERM].transpose(1, 0, 2).reshape(LEAF_DIMS, -1).astype(BF16))
    lwt = np.ascontiguousarray(
        leaf_weight[_LEAF_PERM].transpose(1, 0, 2).reshape(LEAF_DIMS, -1).astype(BF16))
    if use_bias:
        bias = np.ascontiguousarray(b[_NODES_PERM].reshape(1, NT).astype(BF16))

    in_maps = []
    for c in range(N_CORES):
        xs = x[c * BS:(c + 1) * BS]                       # [512, 512]
        xTc = np.ascontiguousarray(xs.T.reshape(KT, 128, BS).astype(BF16))
        m = {"xT": xTc, "Wf": Wf, "gt": gt, "lwt": lwt}
        if use_bias:
            m["bias"] = bias
        in_maps.append(m)
    return use_bias, in_maps


def kernel(x, W, b, leaf_weight, gates):
    from concourse.bass_utils import run_bass_kernel_spmd

    use_bias, in_maps = _make_in_maps(x, W, b, leaf_weight, gates)
    nc = _get_nc(use_bias)

    res = run_bass_kernel_spmd(nc, in_maps, core_ids=list(range(N_CORES)))
    out = np.empty((BATCH, LEAF_DIMS), dtype=np.float32)
    for c in range(N_CORES):
        out[c * BS:(c + 1) * BS] = res.results[c]["outT"].T
    return out



# revision 26
# speedup vs baseline: 1.2947x; 1.2947x over previous
"""MixtureOfExpertsTreeEnsemble Trainium2 kernel (8-core SPMD, batch data-parallel).

Math (per batch row b, tree t):
  g[b,n,t] = sigmoid(x[b] @ W[n,:,t] + bias[n,t])          63 internal nodes
  p[b,l,t] = prod of g / (1-g) along root->leaf path        64 leaves
  w[l,d,t] = leaf_weight[l,d,t] * softmax_t(gates[l,d,t])
  out[b,d] = sum_{l,t} p[b,l,t] * w[l,d,t]

Sharding: batch 4096 -> 8 cores x 512 rows; node weights / leaf tables
replicated.  No collectives; host concatenates the per-core outputs.

Device design notes (per core), "transposed" layout with (node,tree) on
partitions and batch free -- no p-transposes anywhere:
  * phase A (PE): logit tiles [nt(128), b(512)] = W_tile.T @ x, W stationary.
    Node order is bit-reversed per level; level-0 weights appear twice with
    flipped sign so one activation yields both children of the root.  Levels
    0-4 (tiles 0..15) bf16; level 5 (tiles 16..31, half the MACs) can run
    fp8e4 DoubleRow (USE_FP8), but the ISA only allows DoubleRow outputs at
    PSUM partitions 0-63, which clashes with full-partition sigmoid reads,
    so bf16 everywhere is the shipping config.
  * phase B (DVE): level doubling across partition-chunks: left = p*g
    (rows align), right = p - left; all [128, k*512] bf16 tensor_tensor
    ops in the DVE 2x 16-bit mode.
  * phase 0: exp on ACT, tree-sum on DVE, 64 per-leaf normalize STTs on the
    idle Pool engine; wsm -> wT transposed by the DMA xbar engine
    (dma_start_transpose), not the PE.
  * phase D (PE): out[d,b] += wT_chunk.T @ p_chunk over 32 chunks,
    interleaved into phase A's instruction stream as chunks become ready.
  * DMA: everything on the SP ring in explicit consumption order (a
    dma_start blocks its engine's sequencer, and SP has no compute); the
    ACT ring carries only the output store.  PE "filler" transposes chained
    on DMA arrivals keep the Tensor-engine pstate ramp warm through gaps.
"""

import sys

sys.path.insert(0, "/opt/trn_rl_repo")

import ml_dtypes
import numpy as np

BF16 = np.dtype(ml_dtypes.bfloat16)
F8 = np.dtype(ml_dtypes.float8_e4m3fn)

MAX_DEPTH = 6
NUM_TREES = 64
LEAF_DIMS = 128
D_IN = 512
BATCH = 4096
N_CORES = 8
BS = BATCH // N_CORES          # 512 batch rows per core
USE_FP8 = False                 # fp8e4 DoubleRow for level-5 logits
WT_DMA = True                  # wT transposes on the DMA xbar (else PE)


def _bitrev(x: int, bits: int) -> int:
    r = 0
    for _ in range(bits):
        r = (r << 1) | (x & 1)
        x >>= 1
    return r


_NODES_PERM = np.array(
    [(2**lvl - 1) + _bitrev(j, lvl) for lvl in range(MAX_DEPTH) for j in range(2**lvl)]
)
_LEAF_PERM = np.array([_bitrev(j, MAX_DEPTH) for j in range(64)])

_BUILT = {}


DEBUG_DUMP = False


def _build(use_bias: bool, use_fp8: bool):
    import concourse.bacc as bacc
    import concourse.tile as tile
    from concourse import mybir
    from concourse.masks import make_identity

    f32 = mybir.dt.float32
    bf16 = mybir.dt.bfloat16
    fp8 = mybir.dt.float8e4
    AF = mybir.ActivationFunctionType
    AX = mybir.AxisListType
    ADD = mybir.AluOpType.add
    MUL = mybir.AluOpType.mult
    SUB = mybir.AluOpType.subtract
    DR = mybir.MatmulPerfMode.DoubleRow

    NBF = 16 if use_fp8 else 32    # bf16 nt-tiles

    nc = bacc.Bacc("TRN2", target_bir_lowering=False, debug=False)

    xT = nc.dram_tensor("xT", [128, 4, BS], bf16, kind="ExternalInput")
    Wf16 = nc.dram_tensor("Wf16", [128, NBF, 4, 128], bf16, kind="ExternalInput")
    if use_fp8:
        x8 = nc.dram_tensor("x8", [128, 2, 2, BS], fp8, kind="ExternalInput")
        W8f = nc.dram_tensor("W8f", [128, 16, 2, 2, 128], fp8, kind="ExternalInput")
    gt = nc.dram_tensor("gt", [128, 4096], bf16, kind="ExternalInput")
    lwt = nc.dram_tensor("lwt", [128, 4096], bf16, kind="ExternalInput")
    if use_bias:
        bias_d = nc.dram_tensor("bias", [128, 32], f32, kind="ExternalInput")
    outT = nc.dram_tensor("outT", [LEAF_DIMS, BS], f32, kind="ExternalOutput")
    if DEBUG_DUMP:
        dbg_g = nc.dram_tensor("dbg_g", [128, 32 * BS], bf16, kind="ExternalOutput")
        dbg_pf = nc.dram_tensor("dbg_pf", [128, 32 * BS], bf16, kind="ExternalOutput")
        dbg_wsm = nc.dram_tensor("dbg_wsm", [128, 4096], bf16, kind="ExternalOutput")
        dbg_wT = nc.dram_tensor("dbg_wT", [128, 4096], bf16, kind="ExternalOutput")
        dbg_w16 = nc.dram_tensor("dbg_w16", [128, 16384], bf16, kind="ExternalOutput")
        dbg_xk = nc.dram_tensor("dbg_xk", [128, 2048], bf16, kind="ExternalOutput")

    with tile.TileContext(nc) as tc:
        with tc.tile_pool(name="const", bufs=1) as cpool, \
             tc.tile_pool(name="wts", bufs=1) as wpool, \
             tc.tile_pool(name="psA", bufs=2, space="PSUM") as psA, \
             tc.tile_pool(name="psD", bufs=1, space="PSUM") as psD, \
             tc.tile_pool(name="psF", bufs=1, space="PSUM") as psF:

            ident = cpool.tile([128, 128], bf16, tag="ident")
            make_identity(nc, ident[:])

            # ---- SBUF tensors ----
            xk = wpool.tile([128, 4, BS], bf16, tag="xk")
            w16 = wpool.tile([128, NBF, 4, 128], bf16, tag="w16")
            gtile = wpool.tile([128, 64, 64], bf16, tag="gtile")
            lwtile = wpool.tile([128, 64, 64], bf16, tag="lwtile")
            if use_fp8:
                w8 = wpool.tile([128, 16, 2, 2, 128], fp8, tag="w8")
                x8sb = wpool.tile([128, 2, 2, BS], fp8, tag="x8sb")
            g_all = wpool.tile([128, 32, BS], bf16, tag="g")
            st2 = wpool.tile([128, 2, BS], bf16, tag="st2")
            st3 = wpool.tile([128, 4, BS], bf16, tag="st3")
            st4 = wpool.tile([128, 8, BS], bf16, tag="st4")
            st5 = wpool.tile([128, 16, BS], bf16, tag="st5")
            pf = wpool.tile([128, 32, BS], bf16, tag="pf")
            wsm = wpool.tile([128, 64, 64], bf16, tag="wsm")
            elw = wpool.tile([128, 64, 64], bf16, tag="elw")
            wT_all = wpool.tile([128, 8, 4, 128], bf16, tag="wT")
            out_sb = wpool.tile([LEAF_DIMS, BS], f32, tag="out_sb")
            dps = psD.tile([128, BS], f32, tag="dps")

            # ---- input DMAs: all on the SP ring, consumption order ----
            gt3 = gt[:, :].rearrange("d (l t) -> d l t", t=64)
            lwt3 = lwt[:, :].rearrange("d (l t) -> d l t", t=64)
            sp_seq = [
                (w16[:, 0:2, :, :], Wf16[:, 0:2, :, :]),
                (xk[:, :, :], xT[:, :, :]),   # [128, 4, BS], partition-first
                (gtile[:, 0:16, :], gt3[:, 0:16, :]),
                (gtile[:, 16:32, :], gt3[:, 16:32, :]),
                (w16[:, 2:4, :, :], Wf16[:, 2:4, :, :]),
                (gtile[:, 32:48, :], gt3[:, 32:48, :]),
                (gtile[:, 48:64, :], gt3[:, 48:64, :]),
                (w16[:, 4:6, :, :], Wf16[:, 4:6, :, :]),
                (lwtile[:, 0:32, :], lwt3[:, 0:32, :]),
                (w16[:, 6:8, :, :], Wf16[:, 6:8, :, :]),
                (w16[:, 8:10, :, :], Wf16[:, 8:10, :, :]),
            ]
            if use_fp8:
                sp_seq.append((x8sb[:, :, :, :], x8[:, :, :, :]))
            sp_seq += [
                (w16[:, 10:12, :, :], Wf16[:, 10:12, :, :]),
                (w16[:, 12:14, :, :], Wf16[:, 12:14, :, :]),
                (w16[:, 14:16, :, :], Wf16[:, 14:16, :, :]),
                (lwtile[:, 32:64, :], lwt3[:, 32:64, :]),
            ]
            if not use_fp8:
                for pair in range(8, 16):
                    sp_seq.append((w16[:, 2 * pair:2 * pair + 2, :, :],
                                   Wf16[:, 2 * pair:2 * pair + 2, :, :]))
            else:
                for qd in range(4):
                    sp_seq.append((w8[:, 4 * qd:4 * qd + 4, :, :, :],
                                   W8f[:, 4 * qd:4 * qd + 4, :, :, :]))
            for dst, src in sp_seq:
                nc.sync.dma_start(dst, src)
            if use_bias:
                bias_sb = cpool.tile([128, 32], f32, tag="bias")
                nc.sync.dma_start(bias_sb[:], bias_d[:, :])

            # ---- ACT stream: exp quarters (pipelined with the gt DMAs) ----
            for q in range(4):
                nc.scalar.activation(gtile[:, 16 * q:16 * q + 16, :],
                                     gtile[:, 16 * q:16 * q + 16, :], AF.Exp)

            # ---- DVE stream: softmax reduce per leaf half ----
            ehalf = cpool.tile([128, 64, 32], bf16, tag="ehalf")
            equar = cpool.tile([128, 64, 16], bf16, tag="equar")
            ssum = cpool.tile([128, 64], f32, tag="ssum")
            rcp = cpool.tile([128, 64], f32, tag="rcp")

            def emit_smax_reduce(h):
                sl = slice(32 * h, 32 * h + 32)
                nc.vector.tensor_add(ehalf[:, sl, :], gtile[:, sl, 0:32],
                                     gtile[:, sl, 32:64])
                nc.vector.tensor_add(equar[:, sl, :], ehalf[:, sl, 0:16],
                                     ehalf[:, sl, 16:32])
                nc.vector.reduce_sum(ssum[:, sl], equar[:, sl, :], axis=AX.X)
                nc.vector.reciprocal(rcp[:, sl], ssum[:, sl])

            # ---- phase-0 normalize: elw = e*lw on DVE (one 2x-mode op per
            # half), then wsm[l] = elw[l] * r[l] as per-leaf single-op
            # tensor_scalar_mul on the idle Pool engine (the two-op STT form
            # is not legal on Pool). ----
            def emit_elw(h):
                sl = slice(32 * h, 32 * h + 32)
                nc.vector.tensor_mul(elw[:, sl, :], gtile[:, sl, :],
                                     lwtile[:, sl, :])

            def emit_norm(l):
                nc.gpsimd.tensor_scalar_mul(wsm[:, l, :], elw[:, l, :],
                                            rcp[:, l:l + 1])

            # ---- wT: transpose wsm group (8 leaves = 4 chunks) ----
            if WT_DMA:
                def emit_wT(grp):
                    nc.sync.dma_start_transpose(
                        wT_all[:, grp, :, :], wsm[:, 8 * grp:8 * grp + 8, :])
            else:
                def emit_wT(grp):
                    tp = psF.tile([128, 512], bf16, tag="tp", name=f"tp{grp}")
                    for q in range(4):
                        chi = 4 * grp + q
                        nc.tensor.transpose(tp[:, q * 128:(q + 1) * 128],
                                            wsm[:, 2 * chi:2 * chi + 2, :], ident[:])
                    nc.gpsimd.tensor_copy(
                        wT_all[:, grp, :, :].rearrange("p a b -> p (a b)"), tp[:])

            # ---- PE warm-up + DMA-chained fillers (pstate keep-alive) ----
            for wi in range(4):
                warm = psF.tile([128, 512], bf16, tag="tp", name=f"warm{wi}")
                nc.tensor.transpose(warm[:, 0:128], ident[:], ident[:])

            fill_srcs = ([lambda: gtile[:, 0:2, :], lambda: gtile[:, 32:34, :],
                          lambda: lwtile[:, 0:2, :], lambda: lwtile[:, 32:34, :]]
                         + [lambda i=i: w16[:, min(2 * i + 1, NBF - 1), 0, :]
                            for i in range(3, 8)])
            fill_iter = iter(fill_srcs)
            fill_n = [0]

            def emit_filler():
                try:
                    src = next(fill_iter)()
                except StopIteration:
                    return
                fill_n[0] += 1
                warm = psF.tile([128, 512], bf16, tag="tp",
                                name=f"fill{fill_n[0]}")
                nc.tensor.transpose(warm[:, 0:128], src, ident[:])

            # ---- sigmoid over a psA group ----
            def emit_sigmoid(lg, t0, n):
                if use_bias:
                    for u in range(n):
                        nc.scalar.activation(
                            g_all[:, t0 + u, :], lg[:, u, :], AF.Sigmoid,
                            bias=bias_sb[:, t0 + u:t0 + u + 1])
                else:
                    nc.scalar.activation(g_all[:, t0:t0 + n, :], lg[:, 0:n, :],
                                         AF.Sigmoid)

            # ---- phase-B stages (DVE tensor_tensor: 2x bf16 mode) ----
            def emit_stage(s):
                n_par = 2 ** (s - 2)
                gofs = 2 ** (s - 2)
                par = {2: g_all, 3: st2, 4: st3, 5: st4, 6: st5}[s]
                dst = {2: st2, 3: st3, 4: st4, 5: st5, 6: pf}[s]
                psl = par[:, 0:1, :] if s == 2 else par[:, 0:n_par, :]
                nc.vector.tensor_mul(dst[:, 0:n_par, :], psl,
                                     g_all[:, gofs:gofs + n_par, :])
                nc.vector.tensor_sub(dst[:, n_par:2 * n_par, :], psl,
                                     dst[:, 0:n_par, :])

            def emit_stage5_half(hh):
                a, b2 = 4 * hh, 4 * hh + 4
                nc.vector.tensor_mul(st5[:, a:b2, :], st4[:, a:b2, :],
                                     g_all[:, 8 + a:8 + b2, :])
                nc.vector.tensor_sub(st5[:, 8 + a:8 + b2, :], st4[:, a:b2, :],
                                     st5[:, a:b2, :])

            def emit_st6_mul(a, b2):
                nc.vector.tensor_mul(pf[:, a:b2, :], st5[:, a:b2, :],
                                     g_all[:, 16 + a:16 + b2, :])

            def emit_st6_sub(a, b2):
                nc.vector.tensor_sub(pf[:, 16 + a:16 + b2, :], st5[:, a:b2, :],
                                     pf[:, a:b2, :])

            # ---- phase-D matmul for chunk chi ----
            def emit_dmm(chi, start, stop):
                nc.tensor.matmul(dps[:, :], wT_all[:, chi // 4, chi % 4, :],
                                 pf[:, chi, :], start=start, stop=stop)

            # ---- schedule ----
            # sigmoid groups of 3 A-tiles (last group 2); chunk c's sigmoid
            # is emitted at the end of tile 3*(c//3)+2, i.e. after tau_sig(c)
            def tau_sig(c):
                return c if c >= 30 else min(31, 3 * (c // 3) + 2)

            inject = {tau: [] for tau in range(32)}
            for tau in range(1, 10):
                inject[tau].append(emit_filler)
            emit_smax_reduce(0)          # DVE: ready after exp q0-q1, pre-sigmoid window
            inject[tau_sig(1)].append(lambda: emit_stage(2))
            inject[tau_sig(2)].append(lambda: emit_smax_reduce(1))
            inject[tau_sig(1)].append(lambda: emit_elw(0))
            inject[tau_sig(3)].append(lambda: emit_stage(3))
            inject[tau_sig(3)].append(lambda: emit_elw(1))
            inject[tau_sig(7)].append(lambda: emit_stage(4))
            inject[tau_sig(11)].append(lambda: emit_stage5_half(0))
            inject[tau_sig(15)].append(lambda: emit_stage5_half(1))
            # stage 6 + phase D, fine-grained at the tail.  dseq fixes the
            # accumulation order; stop is the last right chunk (31).
            dseq = []
            for m in range(4):
                dseq += list(range(4 * m, 4 * m + 4))
                dseq += list(range(16 + 4 * m, 16 + 4 * m + 4))
            dpos = {chi: (i == 0, i == 31) for i, chi in enumerate(dseq)}

            def emit_dgroup(chis):
                for c in chis:
                    emit_dmm(c, dpos[c][0], dpos[c][1])

            inject[tau_sig(19)].append(lambda: emit_st6_mul(0, 4))
            inject[tau_sig(19)].append(lambda: emit_st6_sub(0, 4))
            inject[min(31, tau_sig(19) + 1)].append(
                lambda: emit_dgroup(dseq[0:8]))
            inject[tau_sig(23)].append(lambda: emit_st6_mul(4, 8))
            inject[tau_sig(23)].append(lambda: emit_st6_sub(4, 8))
            inject[min(31, tau_sig(23) + 1)].append(
                lambda: emit_dgroup(dseq[8:16]))
            # chunks 8..15 per-sigmoid-group granularity for a short tail
            inject[26].append(lambda: emit_st6_mul(8, 11))
            inject[26].append(lambda: emit_st6_sub(8, 11))
            inject[27].append(lambda: emit_dgroup([8, 9, 10, 24, 25, 26]))
            inject[29].append(lambda: emit_st6_mul(11, 14))
            inject[29].append(lambda: emit_st6_sub(11, 14))
            inject[30].append(lambda: emit_dgroup([11, 12, 13, 27, 28, 29]))
            inject[30].append(lambda: emit_st6_mul(14, 15))
            inject[30].append(lambda: emit_st6_sub(14, 15))
            inject[30].append(lambda: emit_dgroup([14, 30]))
            inject[31].append(lambda: emit_st6_mul(15, 16))
            inject[31].append(lambda: emit_st6_sub(15, 16))
            inject[31].append(lambda: emit_dgroup([15, 31]))
            # Pool normalize spread through phase A
            for l in range(64):
                inject[min(31, 4 + l // 6)].append(lambda l=l: emit_norm(l))
            for grp in range(8):
                inject[min(31, 15 + grp)].append(lambda g2=grp: emit_wT(g2))

            lg = None
            t0 = 0
            for tau in range(32):
                gi = tau - t0
                if gi == 0:
                    ntile = 3 if tau < 30 else 1
                    lg = psA.tile([128, 3, 512], f32, tag="lg")
                if tau < NBF:
                    for kk in range(4):
                        nc.tensor.matmul(lg[:, gi, :], w16[:, tau, kk, :],
                                         xk[:, kk, :],
                                         start=(kk == 0), stop=(kk == 3))
                else:
                    taup = tau - 16
                    for h in range(2):
                        for c in range(2):
                            for j in range(2):
                                nc.tensor.matmul(
                                    lg[64 * h:64 * h + 64, gi,
                                       256 * c:256 * c + 256],
                                    w8[:, taup, j, :, 64 * h:64 * h + 64],
                                    x8sb[:, j, :, 256 * c:256 * c + 256],
                                    start=(j == 0), stop=(j == 1),
                                    perf_mode=DR)
                if gi == ntile - 1:
                    emit_sigmoid(lg, t0, ntile)
                    t0 = tau + 1
                for fn in inject[tau]:
                    fn()

            if DEBUG_DUMP:
                nc.scalar.dma_start(
                    dbg_g[:, :], g_all[:, :, :].rearrange("p a b -> p (a b)"))
                nc.scalar.dma_start(
                    dbg_pf[:, :], pf[:, :, :].rearrange("p a b -> p (a b)"))
                nc.scalar.dma_start(
                    dbg_wsm[:, :], wsm[:, :, :].rearrange("p a b -> p (a b)"))
                nc.scalar.dma_start(
                    dbg_wT[:, :],
                    wT_all[:, :, :, :].rearrange("p a b c -> p (a b c)"))
                nc.scalar.dma_start(
                    dbg_w16[:, :],
                    w16[:, :, :, :].rearrange("p a b c -> p (a b c)"))
                nc.scalar.dma_start(
                    dbg_xk[:, :], xk[:, :, :].rearrange("p a b -> p (a b)"))

            # ---- tail: copy + store in halves so DMA overlaps the copy ----
            nc.vector.tensor_copy(out_sb[:, 0:256], dps[:, 0:256])
            nc.sync.dma_start(outT[:, 0:256], out_sb[:, 0:256])
            nc.vector.tensor_copy(out_sb[:, 256:512], dps[:, 256:512])
            nc.sync.dma_start(outT[:, 256:512], out_sb[:, 256:512])

    nc.finalize()
    return nc


def _get_nc(use_bias: bool, use_fp8: bool = USE_FP8):
    key = (use_bias, use_fp8)
    if key not in _BUILT:
        _BUILT[key] = _build(use_bias, use_fp8)
    return _BUILT[key]


def _make_in_maps(x, W, b, leaf_weight, gates, use_fp8):
    x = np.ascontiguousarray(np.asarray(x, dtype=np.float32))
    W = np.asarray(W, dtype=np.float32)
    b = np.asarray(b, dtype=np.float32)
    leaf_weight = np.asarray(leaf_weight, dtype=np.float32)
    gates = np.asarray(gates, dtype=np.float32)

    use_bias = bool(np.any(b))
    Wp = W[_NODES_PERM]                                   # [63, 512, 64]
    W2 = np.concatenate([Wp[0:1], -Wp[0:1], Wp[1:]], axis=0)   # [64, 512, 64]
    Wflat = W2.transpose(1, 0, 2).reshape(D_IN, 4096)     # [k, nt]
    NBF = 16 if use_fp8 else 32
    Wbf = Wflat[:, :NBF * 128].reshape(4, 128, NBF, 128)
    Wf16 = np.ascontiguousarray(Wbf.transpose(1, 2, 0, 3)).astype(BF16)
    shared = {"Wf16": Wf16}
    if use_fp8:
        W8 = Wflat[:, 2048:].reshape(2, 2, 128, 16, 128)
        shared["W8f"] = np.ascontiguousarray(W8.transpose(2, 3, 0, 1, 4)).astype(F8)
    shared["gt"] = np.ascontiguousarray(
        gates[_LEAF_PERM].transpose(1, 0, 2).reshape(128, 4096)).astype(BF16)
    shared["lwt"] = np.ascontiguousarray(
        leaf_weight[_LEAF_PERM].transpose(1, 0, 2).reshape(128, 4096)
    ).astype(BF16)
    if use_bias:
        bp = b[_NODES_PERM]                               # [63, 64]
        b2 = np.concatenate([bp[0:1], -bp[0:1], bp[1:]], axis=0).reshape(4096)
        shared["bias"] = np.ascontiguousarray(
            b2.reshape(32, 128).T.copy()).astype(np.float32)

    in_maps = []
    for c in range(N_CORES):
        xs = x[c * BS:(c + 1) * BS]                       # [512, 512]
        m = dict(shared)
        m["xT"] = np.ascontiguousarray(
            xs.T.reshape(4, 128, BS).transpose(1, 0, 2)).astype(BF16)
        if use_fp8:
            m["x8"] = np.ascontiguousarray(
                xs.T.reshape(2, 2, 128, BS).transpose(2, 0, 1, 3)).astype(F8)
        in_maps.append(m)
    return use_bias, in_maps


def kernel(x, W, b, leaf_weight, gates):
    from concourse.bass_utils import run_bass_kernel_spmd

    use_bias, in_maps = _make_in_maps(x, W, b, leaf_weight, gates, USE_FP8)
    nc = _get_nc(use_bias, USE_FP8)

    res = run_bass_kernel_spmd(nc, in_maps, core_ids=list(range(N_CORES)))
    out = np.empty((BATCH, LEAF_DIMS), dtype=np.float32)
    for c in range(N_CORES):
        out[c * BS:(c + 1) * BS] = res.results[c]["outT"].T
    return out


# revision 33
# speedup vs baseline: 1.3108x; 1.0125x over previous
"""MixtureOfExpertsTreeEnsemble Trainium2 kernel (8-core SPMD, batch data-parallel).

Math (per batch row b, tree t):
  g[b,n,t] = sigmoid(x[b] @ W[n,:,t] + bias[n,t])          63 internal nodes
  p[b,l,t] = prod of g / (1-g) along root->leaf path        64 leaves
  w[l,d,t] = leaf_weight[l,d,t] * softmax_t(gates[l,d,t])
  out[b,d] = sum_{l,t} p[b,l,t] * w[l,d,t]

Sharding: batch 4096 -> 8 cores x 512 rows; node weights / leaf tables
replicated.  No collectives; host concatenates the per-core outputs.

Device design notes (per core), "transposed" layout with (node,tree) on
partitions and batch free -- no p-transposes anywhere:
  * phase A (PE): logit tiles [nt(128), b(512)] = W_tile.T @ x, W stationary.
    Node order is bit-reversed per level; level-0 weights appear twice with
    flipped sign so one activation yields both children of the root.  Levels
    0-4 (tiles 0..15) bf16; level 5 (tiles 16..31, half the MACs) can run
    fp8e4 DoubleRow (USE_FP8), but the ISA only allows DoubleRow outputs at
    PSUM partitions 0-63, which clashes with full-partition sigmoid reads,
    so bf16 everywhere is the shipping config.
  * phase B (DVE): level doubling across partition-chunks: left = p*g
    (rows align), right = p - left; all [128, k*512] bf16 tensor_tensor
    ops in the DVE 2x 16-bit mode.
  * phase 0: exp on ACT, tree-sum on DVE, 64 per-leaf normalize STTs on the
    idle Pool engine; wsm -> wT transposed by the DMA xbar engine
    (dma_start_transpose), not the PE.
  * phase D (PE): out[d,b] += wT_chunk.T @ p_chunk over 32 chunks,
    interleaved into phase A's instruction stream as chunks become ready.
  * DMA: everything on the SP ring in explicit consumption order (a
    dma_start blocks its engine's sequencer, and SP has no compute); the
    ACT ring carries only the output store.  PE "filler" transposes chained
    on DMA arrivals keep the Tensor-engine pstate ramp warm through gaps.
"""

import sys

sys.path.insert(0, "/opt/trn_rl_repo")

import ml_dtypes
import numpy as np

BF16 = np.dtype(ml_dtypes.bfloat16)
F8 = np.dtype(ml_dtypes.float8_e4m3fn)

MAX_DEPTH = 6
NUM_TREES = 64
LEAF_DIMS = 128
D_IN = 512
BATCH = 4096
N_CORES = 8
BS = BATCH // N_CORES          # 512 batch rows per core
USE_FP8 = False                 # fp8e4 DoubleRow for level-5 logits
WT_DMA = True                  # wT transposes on the DMA xbar (else PE)


def _bitrev(x: int, bits: int) -> int:
    r = 0
    for _ in range(bits):
        r = (r << 1) | (x & 1)
        x >>= 1
    return r


_NODES_PERM = np.array(
    [(2**lvl - 1) + _bitrev(j, lvl) for lvl in range(MAX_DEPTH) for j in range(2**lvl)]
)
_LEAF_PERM = np.array([_bitrev(j, MAX_DEPTH) for j in range(64)])

_BUILT = {}


DEBUG_DUMP = False


def _build(use_bias: bool, use_fp8: bool):
    import concourse.bacc as bacc
    import concourse.tile as tile
    from concourse import mybir
    from concourse.masks import make_identity

    f32 = mybir.dt.float32
    bf16 = mybir.dt.bfloat16
    fp8 = mybir.dt.float8e4
    AF = mybir.ActivationFunctionType
    AX = mybir.AxisListType
    ADD = mybir.AluOpType.add
    MUL = mybir.AluOpType.mult
    SUB = mybir.AluOpType.subtract
    DR = mybir.MatmulPerfMode.DoubleRow

    NBF = 16 if use_fp8 else 32    # bf16 nt-tiles

    nc = bacc.Bacc("TRN2", target_bir_lowering=False, debug=False)

    xT = nc.dram_tensor("xT", [128, 4, BS], bf16, kind="ExternalInput")
    Wf16 = nc.dram_tensor("Wf16", [128, NBF, 4, 128], bf16, kind="ExternalInput")
    if use_fp8:
        x8 = nc.dram_tensor("x8", [128, 2, 2, BS], fp8, kind="ExternalInput")
        W8f = nc.dram_tensor("W8f", [128, 16, 2, 2, 128], fp8, kind="ExternalInput")
    gt = nc.dram_tensor("gt", [128, 4096], bf16, kind="ExternalInput")
    lwt = nc.dram_tensor("lwt", [128, 4096], bf16, kind="ExternalInput")
    if use_bias:
        bias_d = nc.dram_tensor("bias", [128, 32], f32, kind="ExternalInput")
    outT = nc.dram_tensor("outT", [LEAF_DIMS, BS], f32, kind="ExternalOutput")
    if DEBUG_DUMP:
        dbg_g = nc.dram_tensor("dbg_g", [128, 32 * BS], bf16, kind="ExternalOutput")
        dbg_pf = nc.dram_tensor("dbg_pf", [128, 32 * BS], bf16, kind="ExternalOutput")
        dbg_wsm = nc.dram_tensor("dbg_wsm", [128, 4096], bf16, kind="ExternalOutput")
        dbg_wT = nc.dram_tensor("dbg_wT", [128, 4096], bf16, kind="ExternalOutput")
        dbg_w16 = nc.dram_tensor("dbg_w16", [128, 16384], bf16, kind="ExternalOutput")
        dbg_xk = nc.dram_tensor("dbg_xk", [128, 2048], bf16, kind="ExternalOutput")

    with tile.TileContext(nc) as tc:
        with tc.tile_pool(name="const", bufs=1) as cpool, \
             tc.tile_pool(name="wts", bufs=1) as wpool, \
             tc.tile_pool(name="psA", bufs=2, space="PSUM") as psA, \
             tc.tile_pool(name="psD", bufs=1, space="PSUM") as psD, \
             tc.tile_pool(name="psF", bufs=1, space="PSUM") as psF:

            ident = cpool.tile([128, 128], bf16, tag="ident")
            make_identity(nc, ident[:])

            # ---- SBUF tensors ----
            xk = wpool.tile([128, 4, BS], bf16, tag="xk")
            w16 = wpool.tile([128, NBF, 4, 128], bf16, tag="w16")
            gtile = wpool.tile([128, 64, 64], bf16, tag="gtile")
            lwtile = wpool.tile([128, 64, 64], bf16, tag="lwtile")
            if use_fp8:
                w8 = wpool.tile([128, 16, 2, 2, 128], fp8, tag="w8")
                x8sb = wpool.tile([128, 2, 2, BS], fp8, tag="x8sb")
            g_all = wpool.tile([128, 32, BS], bf16, tag="g")
            st2 = wpool.tile([128, 2, BS], bf16, tag="st2")
            st3 = wpool.tile([128, 4, BS], bf16, tag="st3")
            st4 = wpool.tile([128, 8, BS], bf16, tag="st4")
            st5 = wpool.tile([128, 16, BS], bf16, tag="st5")
            pf = wpool.tile([128, 32, BS], bf16, tag="pf")
            wsm = wpool.tile([128, 64, 64], bf16, tag="wsm")
            elw = wpool.tile([128, 64, 64], bf16, tag="elw")
            wT_all = wpool.tile([128, 8, 4, 128], bf16, tag="wT")
            out_sb = wpool.tile([LEAF_DIMS, BS], f32, tag="out_sb")
            dps = psD.tile([128, BS], f32, tag="dps")

            # ---- input DMAs: all on the SP ring, consumption order ----
            gt3 = gt[:, :].rearrange("d (l t) -> d l t", t=64)
            lwt3 = lwt[:, :].rearrange("d (l t) -> d l t", t=64)
            sp_seq = [
                (w16[:, 0:2, :, :], Wf16[:, 0:2, :, :]),
                (xk[:, :, :], xT[:, :, :]),
                (gtile[:, 0:16, :], gt3[:, 0:16, :]),
                (gtile[:, 16:32, :], gt3[:, 16:32, :]),
                (w16[:, 2:4, :, :], Wf16[:, 2:4, :, :]),
                (gtile[:, 32:48, :], gt3[:, 32:48, :]),
                (gtile[:, 48:64, :], gt3[:, 48:64, :]),
                (w16[:, 4:6, :, :], Wf16[:, 4:6, :, :]),
                (lwtile[:, 0:32, :], lwt3[:, 0:32, :]),
                (w16[:, 6:8, :, :], Wf16[:, 6:8, :, :]),
                (w16[:, 8:10, :, :], Wf16[:, 8:10, :, :]),
            ]
            if use_fp8:
                sp_seq.append((x8sb[:, :, :, :], x8[:, :, :, :]))
            sp_seq += [
                (w16[:, 10:12, :, :], Wf16[:, 10:12, :, :]),
                (w16[:, 12:14, :, :], Wf16[:, 12:14, :, :]),
                (w16[:, 14:16, :, :], Wf16[:, 14:16, :, :]),
                (lwtile[:, 32:64, :], lwt3[:, 32:64, :]),
            ]
            if not use_fp8:
                for pair in range(8, 16):
                    sp_seq.append((w16[:, 2 * pair:2 * pair + 2, :, :],
                                   Wf16[:, 2 * pair:2 * pair + 2, :, :]))
            else:
                for qd in range(4):
                    sp_seq.append((w8[:, 4 * qd:4 * qd + 4, :, :, :],
                                   W8f[:, 4 * qd:4 * qd + 4, :, :, :]))
            for dst, src in sp_seq:
                nc.sync.dma_start(dst, src)
            if use_bias:
                bias_sb = cpool.tile([128, 32], f32, tag="bias")
                nc.sync.dma_start(bias_sb[:], bias_d[:, :])

            # ---- ACT stream: exp quarters (pipelined with the gt DMAs) ----
            for q in range(4):
                nc.scalar.activation(gtile[:, 16 * q:16 * q + 16, :],
                                     gtile[:, 16 * q:16 * q + 16, :], AF.Exp)

            # ---- DVE stream: softmax reduce per leaf half ----
            ehalf = cpool.tile([128, 64, 32], bf16, tag="ehalf")
            equar = cpool.tile([128, 64, 16], bf16, tag="equar")
            ssum = cpool.tile([128, 64], f32, tag="ssum")
            rcp = cpool.tile([128, 64], f32, tag="rcp")

            def emit_smax_reduce(h):
                sl = slice(32 * h, 32 * h + 32)
                nc.vector.tensor_add(ehalf[:, sl, :], gtile[:, sl, 0:32],
                                     gtile[:, sl, 32:64])
                nc.vector.tensor_add(equar[:, sl, :], ehalf[:, sl, 0:16],
                                     ehalf[:, sl, 16:32])
                nc.vector.reduce_sum(ssum[:, sl], equar[:, sl, :], axis=AX.X)
                nc.vector.reciprocal(rcp[:, sl], ssum[:, sl])

            # ---- phase-0 normalize: elw = e*lw on DVE (one 2x-mode op per
            # half), then wsm[l] = elw[l] * r[l] as per-leaf single-op
            # tensor_scalar_mul on the idle Pool engine (the two-op STT form
            # is not legal on Pool). ----
            def emit_elw(h):
                sl = slice(32 * h, 32 * h + 32)
                nc.vector.tensor_mul(elw[:, sl, :], gtile[:, sl, :],
                                     lwtile[:, sl, :])

            def emit_norm(l):
                nc.gpsimd.tensor_scalar_mul(wsm[:, l, :], elw[:, l, :],
                                            rcp[:, l:l + 1])

            # ---- wT: transpose wsm group (8 leaves = 4 chunks) ----
            if WT_DMA:
                def emit_wT(grp):
                    nc.sync.dma_start_transpose(
                        wT_all[:, grp, :, :], wsm[:, 8 * grp:8 * grp + 8, :])
            else:
                def emit_wT(grp):
                    tp = psF.tile([128, 512], bf16, tag="tp", name=f"tp{grp}")
                    for q in range(4):
                        chi = 4 * grp + q
                        nc.tensor.transpose(tp[:, q * 128:(q + 1) * 128],
                                            wsm[:, 2 * chi:2 * chi + 2, :], ident[:])
                    nc.gpsimd.tensor_copy(
                        wT_all[:, grp, :, :].rearrange("p a b -> p (a b)"), tp[:])

            # ---- PE warm-up + DMA-chained fillers (pstate keep-alive) ----
            for wi in range(4):
                warm = psF.tile([128, 512], bf16, tag="tp", name=f"warm{wi}")
                nc.tensor.transpose(warm[:, 0:128], ident[:], ident[:])

            fill_srcs = ([lambda: w16[:, 0, 0, :], lambda: xk[:, 0, 0:128],
                          lambda: xk[:, 2, 0:128], lambda: w16[:, 1, 0, :],
                          lambda: gtile[:, 0:2, :], lambda: gtile[:, 32:34, :],
                          lambda: lwtile[:, 0:2, :]]
                         + [lambda i=i: w16[:, min(2 * i + 1, NBF - 1), 0, :]
                            for i in range(3, 8)])
            fill_iter = iter(fill_srcs)
            fill_n = [0]

            def emit_filler():
                try:
                    src = next(fill_iter)()
                except StopIteration:
                    return
                fill_n[0] += 1
                warm = psF.tile([128, 512], bf16, tag="tp",
                                name=f"fill{fill_n[0]}")
                nc.tensor.transpose(warm[:, 0:128], src, ident[:])

            # ---- sigmoid over a psA group ----
            def emit_sigmoid(lg, t0, n):
                if use_bias:
                    for u in range(n):
                        nc.scalar.activation(
                            g_all[:, t0 + u, :], lg[:, u, :], AF.Sigmoid,
                            bias=bias_sb[:, t0 + u:t0 + u + 1])
                else:
                    nc.scalar.activation(g_all[:, t0:t0 + n, :], lg[:, 0:n, :],
                                         AF.Sigmoid)

            # ---- phase-B stages (DVE tensor_tensor: 2x bf16 mode) ----
            def emit_stage(s):
                n_par = 2 ** (s - 2)
                gofs = 2 ** (s - 2)
                par = {2: g_all, 3: st2, 4: st3, 5: st4, 6: st5}[s]
                dst = {2: st2, 3: st3, 4: st4, 5: st5, 6: pf}[s]
                psl = par[:, 0:1, :] if s == 2 else par[:, 0:n_par, :]
                nc.vector.tensor_mul(dst[:, 0:n_par, :], psl,
                                     g_all[:, gofs:gofs + n_par, :])
                nc.vector.tensor_sub(dst[:, n_par:2 * n_par, :], psl,
                                     dst[:, 0:n_par, :])

            def emit_stage5_half(hh):
                a, b2 = 4 * hh, 4 * hh + 4
                nc.vector.tensor_mul(st5[:, a:b2, :], st4[:, a:b2, :],
                                     g_all[:, 8 + a:8 + b2, :])
                nc.vector.tensor_sub(st5[:, 8 + a:8 + b2, :], st4[:, a:b2, :],
                                     st5[:, a:b2, :])

            def emit_st6_mul(a, b2):
                nc.vector.tensor_mul(pf[:, a:b2, :], st5[:, a:b2, :],
                                     g_all[:, 16 + a:16 + b2, :])

            def emit_st6_sub(a, b2):
                nc.vector.tensor_sub(pf[:, 16 + a:16 + b2, :], st5[:, a:b2, :],
                                     pf[:, a:b2, :])

            # ---- phase-D matmul for chunk chi ----
            def emit_dmm(chi, start, stop):
                nc.tensor.matmul(dps[:, :], wT_all[:, chi // 4, chi % 4, :],
                                 pf[:, chi, :], start=start, stop=stop)

            # early fillers: bridge the warmup->first-tile window
            for _ in range(4):
                emit_filler()

            # ---- schedule ----
            # sigmoid groups of 3 A-tiles (last group 2); chunk c's sigmoid
            # is emitted at the end of tile 3*(c//3)+2, i.e. after tau_sig(c)
            def tau_sig(c):
                return c if c >= 30 else min(31, 3 * (c // 3) + 2)

            inject = {tau: [] for tau in range(32)}
            for tau in range(1, 10):
                inject[tau].append(emit_filler)
            emit_smax_reduce(0)          # DVE: ready after exp q0-q1, pre-sigmoid window
            inject[tau_sig(1)].append(lambda: emit_stage(2))
            inject[tau_sig(2)].append(lambda: emit_smax_reduce(1))
            inject[tau_sig(1)].append(lambda: emit_elw(0))
            inject[tau_sig(3)].append(lambda: emit_stage(3))
            inject[tau_sig(3)].append(lambda: emit_elw(1))
            inject[tau_sig(7)].append(lambda: emit_stage(4))
            inject[tau_sig(11)].append(lambda: emit_stage5_half(0))
            inject[tau_sig(15)].append(lambda: emit_stage5_half(1))
            # stage 6 + phase D, fine-grained at the tail.  dseq fixes the
            # accumulation order; stop is the last right chunk (31).
            dseq = []
            for m in range(4):
                dseq += list(range(4 * m, 4 * m + 4))
                dseq += list(range(16 + 4 * m, 16 + 4 * m + 4))
            dpos = {chi: (i == 0, i == 31) for i, chi in enumerate(dseq)}

            def emit_dgroup(chis):
                for c in chis:
                    emit_dmm(c, dpos[c][0], dpos[c][1])

            inject[tau_sig(19)].append(lambda: emit_st6_mul(0, 4))
            inject[tau_sig(19)].append(lambda: emit_st6_sub(0, 4))

            inject[tau_sig(23)].append(lambda: emit_st6_mul(4, 8))
            inject[tau_sig(23)].append(lambda: emit_st6_sub(4, 8))

            # chunks 8..15 per-sigmoid-group granularity for a short tail
            inject[26].append(lambda: emit_st6_mul(8, 11))
            inject[26].append(lambda: emit_st6_sub(8, 11))

            inject[29].append(lambda: emit_st6_mul(11, 14))
            inject[29].append(lambda: emit_st6_sub(11, 14))

            inject[30].append(lambda: emit_st6_mul(14, 15))
            inject[30].append(lambda: emit_st6_sub(14, 15))

            inject[31].append(lambda: emit_st6_mul(15, 16))
            inject[31].append(lambda: emit_st6_sub(15, 16))

            # Pool normalize spread through phase A
            for l in range(64):
                inject[min(31, 4 + l // 6)].append(lambda l=l: emit_norm(l))
            for grp in range(8):
                inject[min(31, 15 + grp)].append(lambda g2=grp: emit_wT(g2))

            lg = None
            t0 = 0
            for tau in range(32):
                gi = tau - t0
                if gi == 0:
                    ntile = 3 if tau < 30 else 1
                    lg = psA.tile([128, 3, 512], f32, tag="lg")
                if tau < NBF:
                    for kk in range(4):
                        nc.tensor.matmul(lg[:, gi, :], w16[:, tau, kk, :],
                                         xk[:, kk, :],
                                         start=(kk == 0), stop=(kk == 3))
                else:
                    taup = tau - 16
                    for h in range(2):
                        for c in range(2):
                            for j in range(2):
                                nc.tensor.matmul(
                                    lg[64 * h:64 * h + 64, gi,
                                       256 * c:256 * c + 256],
                                    w8[:, taup, j, :, 64 * h:64 * h + 64],
                                    x8sb[:, j, :, 256 * c:256 * c + 256],
                                    start=(j == 0), stop=(j == 1),
                                    perf_mode=DR)
                if gi == ntile - 1:
                    emit_sigmoid(lg, t0, ntile)
                    t0 = tau + 1
                for fn in inject[tau]:
                    fn()

            if DEBUG_DUMP:
                nc.scalar.dma_start(
                    dbg_g[:, :], g_all[:, :, :].rearrange("p a b -> p (a b)"))
                nc.scalar.dma_start(
                    dbg_pf[:, :], pf[:, :, :].rearrange("p a b -> p (a b)"))
                nc.scalar.dma_start(
                    dbg_wsm[:, :], wsm[:, :, :].rearrange("p a b -> p (a b)"))
                nc.scalar.dma_start(
                    dbg_wT[:, :],
                    wT_all[:, :, :, :].rearrange("p a b c -> p (a b c)"))
                nc.scalar.dma_start(
                    dbg_w16[:, :],
                    w16[:, :, :, :].rearrange("p a b c -> p (a b c)"))
                nc.scalar.dma_start(
                    dbg_xk[:, :], xk[:, :, :].rearrange("p a b -> p (a b)"))

            # ---- phase D: all matmuls after the A-stream; they fill the
            # PE while the sigmoid/DVE pipeline drains ----
            emit_dgroup(dseq)

            # ---- tail: copy + store in halves so DMA overlaps the copy ----
            nc.vector.tensor_copy(out_sb[:, 0:256], dps[:, 0:256])
            nc.scalar.dma_start(outT[:, 0:256], out_sb[:, 0:256])
            nc.vector.tensor_copy(out_sb[:, 256:512], dps[:, 256:512])
            nc.scalar.dma_start(outT[:, 256:512], out_sb[:, 256:512])

    nc.finalize()
    return nc


def _get_nc(use_bias: bool, use_fp8: bool = USE_FP8):
    key = (use_bias, use_fp8)
    if key not in _BUILT:
        _BUILT[key] = _build(use_bias, use_fp8)
    return _BUILT[key]


def _make_in_maps(x, W, b, leaf_weight, gates, use_fp8):
    x = np.ascontiguousarray(np.asarray(x, dtype=np.float32))
    W = np.asarray(W, dtype=np.float32)
    b = np.asarray(b, dtype=np.float32)
    leaf_weight = np.asarray(leaf_weight, dtype=np.float32)
    gates = np.asarray(gates, dtype=np.float32)

    use_bias = bool(np.any(b))
    Wp = W[_NODES_PERM]                                   # [63, 512, 64]
    W2 = np.concatenate([Wp[0:1], -Wp[0:1], Wp[1:]], axis=0)   # [64, 512, 64]
    Wflat = W2.transpose(1, 0, 2).reshape(D_IN, 4096)     # [k, nt]
    NBF = 16 if use_fp8 else 32
    Wbf = Wflat[:, :NBF * 128].reshape(4, 128, NBF, 128)
    Wf16 = np.ascontiguousarray(Wbf.transpose(1, 2, 0, 3)).astype(BF16)
    shared = {"Wf16": Wf16}
    if use_fp8:
        W8 = Wflat[:, 2048:].reshape(2, 2, 128, 16, 128)
        shared["W8f"] = np.ascontiguousarray(W8.transpose(2, 3, 0, 1, 4)).astype(F8)
    shared["gt"] = np.ascontiguousarray(
        gates[_LEAF_PERM].transpose(1, 0, 2).reshape(128, 4096)).astype(BF16)
    shared["lwt"] = np.ascontiguousarray(
        leaf_weight[_LEAF_PERM].transpose(1, 0, 2).reshape(128, 4096)
    ).astype(BF16)
    if use_bias:
        bp = b[_NODES_PERM]                               # [63, 64]
        b2 = np.concatenate([bp[0:1], -bp[0:1], bp[1:]], axis=0).reshape(4096)
        shared["bias"] = np.ascontiguousarray(
            b2.reshape(32, 128).T.copy()).astype(np.float32)

    in_maps = []
    for c in range(N_CORES):
        xs = x[c * BS:(c + 1) * BS]                       # [512, 512]
        m = dict(shared)
        m["xT"] = np.ascontiguousarray(
            xs.T.reshape(4, 128, BS).transpose(1, 0, 2)).astype(BF16)
        if use_fp8:
            m["x8"] = np.ascontiguousarray(
                xs.T.reshape(2, 2, 128, BS).transpose(2, 0, 1, 3)).astype(F8)
        in_maps.append(m)
    return use_bias, in_maps


def kernel(x, W, b, leaf_weight, gates):
    from concourse.bass_utils import run_bass_kernel_spmd

    use_bias, in_maps = _make_in_maps(x, W, b, leaf_weight, gates, USE_FP8)
    nc = _get_nc(use_bias, USE_FP8)

    res = run_bass_kernel_spmd(nc, in_maps, core_ids=list(range(N_CORES)))
    out = np.empty((BATCH, LEAF_DIMS), dtype=np.float32)
    for c in range(N_CORES):
        out[c * BS:(c + 1) * BS] = res.results[c]["outT"].T
    return out


# revision 39
# speedup vs baseline: 1.3116x; 1.0006x over previous
"""MixtureOfExpertsTreeEnsemble Trainium2 kernel (8-core SPMD, batch data-parallel).

Math (per batch row b, tree t):
  g[b,n,t] = sigmoid(x[b] @ W[n,:,t] + bias[n,t])          63 internal nodes
  p[b,l,t] = prod of g / (1-g) along root->leaf path        64 leaves
  w[l,d,t] = leaf_weight[l,d,t] * softmax_t(gates[l,d,t])
  out[b,d] = sum_{l,t} p[b,l,t] * w[l,d,t]

Sharding: batch 4096 -> 8 cores x 512 rows; node weights / leaf tables
replicated.  No collectives; host concatenates the per-core outputs.

Device design notes (per core), "transposed" layout with (node,tree) on
partitions and batch free -- no p-transposes anywhere:
  * phase A (PE): logit tiles [nt(128), b(512)] = W_tile.T @ x, W stationary.
    Node order is bit-reversed per level; level-0 weights appear twice with
    flipped sign so one activation yields both children of the root.  Levels
    0-4 (tiles 0..15) bf16; level 5 (tiles 16..31, half the MACs) can run
    fp8e4 DoubleRow (USE_FP8), but the ISA only allows DoubleRow outputs at
    PSUM partitions 0-63, which clashes with full-partition sigmoid reads,
    so bf16 everywhere is the shipping config.
  * phase B (DVE): level doubling across partition-chunks: left = p*g
    (rows align), right = p - left; all [128, k*512] bf16 tensor_tensor
    ops in the DVE 2x 16-bit mode.
  * phase 0: exp on ACT, tree-sum on DVE, 64 per-leaf normalize STTs on the
    idle Pool engine; wsm -> wT transposed by the DMA xbar engine
    (dma_start_transpose), not the PE.
  * phase D (PE): out[d,b] += wT_chunk.T @ p_chunk over 32 chunks,
    interleaved into phase A's instruction stream as chunks become ready.
  * DMA: everything on the SP ring in explicit consumption order (a
    dma_start blocks its engine's sequencer, and SP has no compute); the
    ACT ring carries only the output store.  PE "filler" transposes chained
    on DMA arrivals keep the Tensor-engine pstate ramp warm through gaps.
"""

import sys

sys.path.insert(0, "/opt/trn_rl_repo")

import ml_dtypes
import numpy as np

BF16 = np.dtype(ml_dtypes.bfloat16)
F8 = np.dtype(ml_dtypes.float8_e4m3fn)

MAX_DEPTH = 6
NUM_TREES = 64
LEAF_DIMS = 128
D_IN = 512
BATCH = 4096
N_CORES = 8
BS = BATCH // N_CORES          # 512 batch rows per core
USE_FP8 = False                 # fp8e4 DoubleRow for level-5 logits
WT_DMA = True                  # wT transposes on the DMA xbar (else PE)


def _bitrev(x: int, bits: int) -> int:
    r = 0
    for _ in range(bits):
        r = (r << 1) | (x & 1)
        x >>= 1
    return r


_NODES_PERM = np.array(
    [(2**lvl - 1) + _bitrev(j, lvl) for lvl in range(MAX_DEPTH) for j in range(2**lvl)]
)
_LEAF_PERM = np.array([_bitrev(j, MAX_DEPTH) for j in range(64)])

_BUILT = {}


DEBUG_DUMP = False


def _build(use_bias: bool, use_fp8: bool):
    import concourse.bacc as bacc
    import concourse.tile as tile
    from concourse import mybir
    from concourse.masks import make_identity

    f32 = mybir.dt.float32
    bf16 = mybir.dt.bfloat16
    fp8 = mybir.dt.float8e4
    AF = mybir.ActivationFunctionType
    AX = mybir.AxisListType
    ADD = mybir.AluOpType.add
    MUL = mybir.AluOpType.mult
    SUB = mybir.AluOpType.subtract
    DR = mybir.MatmulPerfMode.DoubleRow

    NBF = 16 if use_fp8 else 32    # bf16 nt-tiles

    nc = bacc.Bacc("TRN2", target_bir_lowering=False, debug=False)

    xT = nc.dram_tensor("xT", [128, 4, BS], bf16, kind="ExternalInput")
    Wf16 = nc.dram_tensor("Wf16", [128, NBF, 4, 128], bf16, kind="ExternalInput")
    if use_fp8:
        x8 = nc.dram_tensor("x8", [128, 2, 2, BS], fp8, kind="ExternalInput")
        W8f = nc.dram_tensor("W8f", [128, 16, 2, 2, 128], fp8, kind="ExternalInput")
    gt = nc.dram_tensor("gt", [128, 4096], bf16, kind="ExternalInput")
    lwt = nc.dram_tensor("lwt", [128, 4096], bf16, kind="ExternalInput")
    if use_bias:
        bias_d = nc.dram_tensor("bias", [128, 32], f32, kind="ExternalInput")
    outT = nc.dram_tensor("outT", [LEAF_DIMS, BS], f32, kind="ExternalOutput")
    if DEBUG_DUMP:
        dbg_g = nc.dram_tensor("dbg_g", [128, 32 * BS], bf16, kind="ExternalOutput")
        dbg_pf = nc.dram_tensor("dbg_pf", [128, 32 * BS], bf16, kind="ExternalOutput")
        dbg_wsm = nc.dram_tensor("dbg_wsm", [128, 4096], bf16, kind="ExternalOutput")
        dbg_wT = nc.dram_tensor("dbg_wT", [128, 4096], bf16, kind="ExternalOutput")
        dbg_w16 = nc.dram_tensor("dbg_w16", [128, 16384], bf16, kind="ExternalOutput")
        dbg_xk = nc.dram_tensor("dbg_xk", [128, 2048], bf16, kind="ExternalOutput")

    with tile.TileContext(nc) as tc:
        with tc.tile_pool(name="const", bufs=1) as cpool, \
             tc.tile_pool(name="wts", bufs=1) as wpool, \
             tc.tile_pool(name="psA", bufs=2, space="PSUM") as psA, \
             tc.tile_pool(name="psD", bufs=1, space="PSUM") as psD, \
             tc.tile_pool(name="psF", bufs=1, space="PSUM") as psF:

            ident = cpool.tile([128, 128], bf16, tag="ident")
            make_identity(nc, ident[:])

            # ---- SBUF tensors ----
            xk = wpool.tile([128, 4, BS], bf16, tag="xk")
            w16 = wpool.tile([128, NBF, 4, 128], bf16, tag="w16")
            gtile = wpool.tile([128, 64, 64], bf16, tag="gtile")
            lwtile = wpool.tile([128, 64, 64], bf16, tag="lwtile")
            if use_fp8:
                w8 = wpool.tile([128, 16, 2, 2, 128], fp8, tag="w8")
                x8sb = wpool.tile([128, 2, 2, BS], fp8, tag="x8sb")
            g_all = wpool.tile([128, 32, BS], bf16, tag="g")
            st2 = wpool.tile([128, 2, BS], bf16, tag="st2")
            st3 = wpool.tile([128, 4, BS], bf16, tag="st3")
            st4 = wpool.tile([128, 8, BS], bf16, tag="st4")
            st5 = wpool.tile([128, 16, BS], bf16, tag="st5")
            pf = wpool.tile([128, 32, BS], bf16, tag="pf")
            wsm = wpool.tile([128, 64, 64], bf16, tag="wsm")
            elw = wpool.tile([128, 64, 64], bf16, tag="elw")
            wT_all = wpool.tile([128, 8, 4, 128], bf16, tag="wT")
            out_sb = wpool.tile([LEAF_DIMS, BS], f32, tag="out_sb")
            dps = psD.tile([128, BS], f32, tag="dps")

            # ---- input DMAs: all on the SP ring, consumption order ----
            gt3 = gt[:, :].rearrange("d (l t) -> d l t", t=64)
            lwt3 = lwt[:, :].rearrange("d (l t) -> d l t", t=64)
            sp_seq = [
                (w16[:, 0:2, :, :], Wf16[:, 0:2, :, :]),
                (xk[:, :, :], xT[:, :, :]),
                (gtile[:, 0:16, :], gt3[:, 0:16, :]),
                (gtile[:, 16:32, :], gt3[:, 16:32, :]),
                (w16[:, 2:4, :, :], Wf16[:, 2:4, :, :]),
                (gtile[:, 32:48, :], gt3[:, 32:48, :]),
                (gtile[:, 48:64, :], gt3[:, 48:64, :]),
                (w16[:, 4:6, :, :], Wf16[:, 4:6, :, :]),
                (w16[:, 6:8, :, :], Wf16[:, 6:8, :, :]),
                (w16[:, 8:10, :, :], Wf16[:, 8:10, :, :]),
                (lwtile[:, 0:32, :], lwt3[:, 0:32, :]),
            ]
            if use_fp8:
                sp_seq.append((x8sb[:, :, :, :], x8[:, :, :, :]))
            sp_seq += [
                (w16[:, 10:12, :, :], Wf16[:, 10:12, :, :]),
                (w16[:, 12:14, :, :], Wf16[:, 12:14, :, :]),
                (w16[:, 14:16, :, :], Wf16[:, 14:16, :, :]),
                (lwtile[:, 32:64, :], lwt3[:, 32:64, :]),
            ]
            if not use_fp8:
                for pair in range(8, 16):
                    sp_seq.append((w16[:, 2 * pair:2 * pair + 2, :, :],
                                   Wf16[:, 2 * pair:2 * pair + 2, :, :]))
            else:
                for qd in range(4):
                    sp_seq.append((w8[:, 4 * qd:4 * qd + 4, :, :, :],
                                   W8f[:, 4 * qd:4 * qd + 4, :, :, :]))
            for dst, src in sp_seq:
                nc.sync.dma_start(dst, src)
            if use_bias:
                bias_sb = cpool.tile([128, 32], f32, tag="bias")
                nc.sync.dma_start(bias_sb[:], bias_d[:, :])

            # ---- ACT stream: exp quarters (pipelined with the gt DMAs) ----
            for q in range(4):
                nc.scalar.activation(gtile[:, 16 * q:16 * q + 16, :],
                                     gtile[:, 16 * q:16 * q + 16, :], AF.Exp)

            # ---- DVE stream: softmax reduce per leaf half ----
            ehalf = cpool.tile([128, 64, 32], bf16, tag="ehalf")
            equar = cpool.tile([128, 64, 16], bf16, tag="equar")
            ssum = cpool.tile([128, 64], f32, tag="ssum")
            rcp = cpool.tile([128, 64], f32, tag="rcp")

            def emit_smax_reduce(h):
                sl = slice(32 * h, 32 * h + 32)
                nc.vector.tensor_add(ehalf[:, sl, :], gtile[:, sl, 0:32],
                                     gtile[:, sl, 32:64])
                nc.vector.tensor_add(equar[:, sl, :], ehalf[:, sl, 0:16],
                                     ehalf[:, sl, 16:32])
                nc.vector.reduce_sum(ssum[:, sl], equar[:, sl, :], axis=AX.X)
                nc.vector.reciprocal(rcp[:, sl], ssum[:, sl])

            # ---- phase-0 normalize: elw = e*lw on DVE (one 2x-mode op per
            # half), then wsm[l] = elw[l] * r[l] as per-leaf single-op
            # tensor_scalar_mul on the idle Pool engine (the two-op STT form
            # is not legal on Pool). ----
            def emit_elw(h):
                sl = slice(32 * h, 32 * h + 32)
                nc.vector.tensor_mul(elw[:, sl, :], gtile[:, sl, :],
                                     lwtile[:, sl, :])

            def emit_norm(l):
                nc.gpsimd.tensor_scalar_mul(wsm[:, l, :], elw[:, l, :],
                                            rcp[:, l:l + 1])

            # ---- wT: transpose wsm group (8 leaves = 4 chunks) ----
            if WT_DMA:
                def emit_wT(grp):
                    nc.sync.dma_start_transpose(
                        wT_all[:, grp, :, :], wsm[:, 8 * grp:8 * grp + 8, :])
            else:
                def emit_wT(grp):
                    tp = psF.tile([128, 512], bf16, tag="tp", name=f"tp{grp}")
                    for q in range(4):
                        chi = 4 * grp + q
                        nc.tensor.transpose(tp[:, q * 128:(q + 1) * 128],
                                            wsm[:, 2 * chi:2 * chi + 2, :], ident[:])
                    nc.gpsimd.tensor_copy(
                        wT_all[:, grp, :, :].rearrange("p a b -> p (a b)"), tp[:])

            # ---- PE warm-up + DMA-chained fillers (pstate keep-alive) ----
            for wi in range(4):
                warm = psF.tile([128, 512], bf16, tag="tp", name=f"warm{wi}")
                nc.tensor.transpose(warm[:, 0:128], ident[:], ident[:])

            fill_srcs = ([lambda: w16[:, 0, 0, :], lambda: xk[:, 0, 0:128],
                          lambda: xk[:, 2, 0:128], lambda: w16[:, 1, 0, :],
                          lambda: gtile[:, 0:2, :], lambda: gtile[:, 32:34, :],
                          lambda: lwtile[:, 0:2, :]]
                         + [lambda i=i: w16[:, min(2 * i + 1, NBF - 1), 0, :]
                            for i in range(3, 8)])
            fill_iter = iter(fill_srcs)
            fill_n = [0]

            def emit_filler():
                try:
                    src = next(fill_iter)()
                except StopIteration:
                    return
                fill_n[0] += 1
                warm = psF.tile([128, 512], bf16, tag="tp",
                                name=f"fill{fill_n[0]}")
                nc.tensor.transpose(warm[:, 0:128], src, ident[:])

            # ---- sigmoid over a psA group ----
            def emit_sigmoid(lg, t0, n):
                if use_bias:
                    for u in range(n):
                        nc.scalar.activation(
                            g_all[:, t0 + u, :], lg[:, u, :], AF.Sigmoid,
                            bias=bias_sb[:, t0 + u:t0 + u + 1])
                else:
                    nc.scalar.activation(g_all[:, t0:t0 + n, :], lg[:, 0:n, :],
                                         AF.Sigmoid)

            # ---- phase-B stages (DVE tensor_tensor: 2x bf16 mode) ----
            def emit_stage(s):
                n_par = 2 ** (s - 2)
                gofs = 2 ** (s - 2)
                par = {2: g_all, 3: st2, 4: st3, 5: st4, 6: st5}[s]
                dst = {2: st2, 3: st3, 4: st4, 5: st5, 6: pf}[s]
                psl = par[:, 0:1, :] if s == 2 else par[:, 0:n_par, :]
                nc.vector.tensor_mul(dst[:, 0:n_par, :], psl,
                                     g_all[:, gofs:gofs + n_par, :])
                nc.vector.tensor_sub(dst[:, n_par:2 * n_par, :], psl,
                                     dst[:, 0:n_par, :])

            def emit_stage5_half(hh):
                a, b2 = 4 * hh, 4 * hh + 4
                nc.vector.tensor_mul(st5[:, a:b2, :], st4[:, a:b2, :],
                                     g_all[:, 8 + a:8 + b2, :])
                nc.vector.tensor_sub(st5[:, 8 + a:8 + b2, :], st4[:, a:b2, :],
                                     st5[:, a:b2, :])

            def emit_st6_mul(a, b2):
                nc.vector.tensor_mul(pf[:, a:b2, :], st5[:, a:b2, :],
                                     g_all[:, 16 + a:16 + b2, :])

            def emit_st6_sub(a, b2):
                nc.vector.tensor_sub(pf[:, 16 + a:16 + b2, :], st5[:, a:b2, :],
                                     pf[:, a:b2, :])

            # ---- phase-D matmul for chunk chi ----
            def emit_dmm(chi, start, stop):
                nc.tensor.matmul(dps[:, :], wT_all[:, chi // 4, chi % 4, :],
                                 pf[:, chi, :], start=start, stop=stop)

            # early fillers: bridge the warmup->first-tile window
            for _ in range(4):
                emit_filler()

            # ---- schedule ----
            # sigmoid groups of 3 A-tiles (last group 2); chunk c's sigmoid
            # is emitted at the end of tile 3*(c//3)+2, i.e. after tau_sig(c)
            def tau_sig(c):
                return c if c >= 30 else min(31, 3 * (c // 3) + 2)

            inject = {tau: [] for tau in range(32)}
            for tau in range(1, 10):
                inject[tau].append(emit_filler)
            emit_smax_reduce(0)          # DVE: ready after exp q0-q1, pre-sigmoid window
            inject[tau_sig(1)].append(lambda: emit_stage(2))
            inject[tau_sig(2)].append(lambda: emit_smax_reduce(1))
            inject[tau_sig(1)].append(lambda: emit_elw(0))
            inject[tau_sig(3)].append(lambda: emit_stage(3))
            inject[tau_sig(3)].append(lambda: emit_elw(1))
            inject[tau_sig(7)].append(lambda: emit_stage(4))
            inject[tau_sig(11)].append(lambda: emit_stage5_half(0))
            inject[tau_sig(15)].append(lambda: emit_stage5_half(1))
            # stage 6 + phase D, fine-grained at the tail.  dseq fixes the
            # accumulation order; stop is the last right chunk (31).
            dseq = []
            for m in range(4):
                dseq += list(range(4 * m, 4 * m + 4))
                dseq += list(range(16 + 4 * m, 16 + 4 * m + 4))
            dpos = {chi: (i == 0, i == 31) for i, chi in enumerate(dseq)}

            def emit_dgroup(chis):
                for c in chis:
                    emit_dmm(c, dpos[c][0], dpos[c][1])

            inject[tau_sig(19)].append(lambda: emit_st6_mul(0, 4))
            inject[tau_sig(19)].append(lambda: emit_st6_sub(0, 4))

            inject[tau_sig(23)].append(lambda: emit_st6_mul(4, 8))
            inject[tau_sig(23)].append(lambda: emit_st6_sub(4, 8))

            # chunks 8..15 per-sigmoid-group granularity for a short tail
            inject[26].append(lambda: emit_st6_mul(8, 11))
            inject[26].append(lambda: emit_st6_sub(8, 11))

            inject[29].append(lambda: emit_st6_mul(11, 14))
            inject[29].append(lambda: emit_st6_sub(11, 14))

            inject[30].append(lambda: emit_st6_mul(14, 15))
            inject[30].append(lambda: emit_st6_sub(14, 15))

            inject[31].append(lambda: emit_st6_mul(15, 16))
            inject[31].append(lambda: emit_st6_sub(15, 16))

            # Pool normalize spread through phase A
            for l in range(64):
                inject[min(31, 4 + l // 6)].append(lambda l=l: emit_norm(l))
            for grp in range(8):
                inject[min(31, 15 + grp)].append(lambda g2=grp: emit_wT(g2))

            lg = None
            t0 = 0
            for tau in range(32):
                gi = tau - t0
                if gi == 0:
                    ntile = 3 if tau < 30 else 1
                    lg = psA.tile([128, 3, 512], f32, tag="lg")
                if tau < NBF:
                    for kk in range(4):
                        nc.tensor.matmul(lg[:, gi, :], w16[:, tau, kk, :],
                                         xk[:, kk, :],
                                         start=(kk == 0), stop=(kk == 3))
                else:
                    taup = tau - 16
                    for h in range(2):
                        for c in range(2):
                            for j in range(2):
                                nc.tensor.matmul(
                                    lg[64 * h:64 * h + 64, gi,
                                       256 * c:256 * c + 256],
                                    w8[:, taup, j, :, 64 * h:64 * h + 64],
                                    x8sb[:, j, :, 256 * c:256 * c + 256],
                                    start=(j == 0), stop=(j == 1),
                                    perf_mode=DR)
                if gi == ntile - 1:
                    emit_sigmoid(lg, t0, ntile)
                    t0 = tau + 1
                for fn in inject[tau]:
                    fn()

            if DEBUG_DUMP:
                nc.scalar.dma_start(
                    dbg_g[:, :], g_all[:, :, :].rearrange("p a b -> p (a b)"))
                nc.scalar.dma_start(
                    dbg_pf[:, :], pf[:, :, :].rearrange("p a b -> p (a b)"))
                nc.scalar.dma_start(
                    dbg_wsm[:, :], wsm[:, :, :].rearrange("p a b -> p (a b)"))
                nc.scalar.dma_start(
                    dbg_wT[:, :],
                    wT_all[:, :, :, :].rearrange("p a b c -> p (a b c)"))
                nc.scalar.dma_start(
                    dbg_w16[:, :],
                    w16[:, :, :, :].rearrange("p a b c -> p (a b c)"))
                nc.scalar.dma_start(
                    dbg_xk[:, :], xk[:, :, :].rearrange("p a b -> p (a b)"))

            # ---- phase D: all matmuls after the A-stream; they fill the
            # PE while the sigmoid/DVE pipeline drains ----
            emit_dgroup(dseq)

            # ---- tail: copy + store in halves so DMA overlaps the copy ----
            nc.vector.tensor_copy(out_sb[:, 0:256], dps[:, 0:256])
            nc.scalar.copy(out_sb[:, 256:512], dps[:, 256:512])
            nc.scalar.dma_start(outT[:, 256:512], out_sb[:, 256:512])
            nc.sync.dma_start(outT[:, 0:256], out_sb[:, 0:256])

    nc.finalize()
    return nc


def _get_nc(use_bias: bool, use_fp8: bool = USE_FP8):
    key = (use_bias, use_fp8)
    if key not in _BUILT:
        _BUILT[key] = _build(use_bias, use_fp8)
    return _BUILT[key]


def _make_in_maps(x, W, b, leaf_weight, gates, use_fp8):
    x = np.ascontiguousarray(np.asarray(x, dtype=np.float32))
    W = np.asarray(W, dtype=np.float32)
    b = np.asarray(b, dtype=np.float32)
    leaf_weight = np.asarray(leaf_weight, dtype=np.float32)
    gates = np.asarray(gates, dtype=np.float32)

    use_bias = bool(np.any(b))
    Wp = W[_NODES_PERM]                                   # [63, 512, 64]
    W2 = np.concatenate([Wp[0:1], -Wp[0:1], Wp[1:]], axis=0)   # [64, 512, 64]
    Wflat = W2.transpose(1, 0, 2).reshape(D_IN, 4096)     # [k, nt]
    NBF = 16 if use_fp8 else 32
    Wbf = Wflat[:, :NBF * 128].reshape(4, 128, NBF, 128)
    Wf16 = np.ascontiguousarray(Wbf.transpose(1, 2, 0, 3)).astype(BF16)
    shared = {"Wf16": Wf16}
    if use_fp8:
        W8 = Wflat[:, 2048:].reshape(2, 2, 128, 16, 128)
        shared["W8f"] = np.ascontiguousarray(W8.transpose(2, 3, 0, 1, 4)).astype(F8)
    shared["gt"] = np.ascontiguousarray(
        gates[_LEAF_PERM].transpose(1, 0, 2).reshape(128, 4096)).astype(BF16)
    shared["lwt"] = np.ascontiguousarray(
        leaf_weight[_LEAF_PERM].transpose(1, 0, 2).reshape(128, 4096)
    ).astype(BF16)
    if use_bias:
        bp = b[_NODES_PERM]                               # [63, 64]
        b2 = np.concatenate([bp[0:1], -bp[0:1], bp[1:]], axis=0).reshape(4096)
        shared["bias"] = np.ascontiguousarray(
            b2.reshape(32, 128).T.copy()).astype(np.float32)

    in_maps = []
    for c in range(N_CORES):
        xs = x[c * BS:(c + 1) * BS]                       # [512, 512]
        m = dict(shared)
        m["xT"] = np.ascontiguousarray(
            xs.T.reshape(4, 128, BS).transpose(1, 0, 2)).astype(BF16)
        if use_fp8:
            m["x8"] = np.ascontiguousarray(
                xs.T.reshape(2, 2, 128, BS).transpose(2, 0, 1, 3)).astype(F8)
        in_maps.append(m)
    return use_bias, in_maps


def kernel(x, W, b, leaf_weight, gates):
    from concourse.bass_utils import run_bass_kernel_spmd

    use_bias, in_maps = _make_in_maps(x, W, b, leaf_weight, gates, USE_FP8)
    nc = _get_nc(use_bias, USE_FP8)

    res = run_bass_kernel_spmd(nc, in_maps, core_ids=list(range(N_CORES)))
    out = np.empty((BATCH, LEAF_DIMS), dtype=np.float32)
    for c in range(N_CORES):
        out[c * BS:(c + 1) * BS] = res.results[c]["outT"].T
    return out
